# revision 23
# baseline (speedup 1.0000x reference)
"""LSH attention on 8 trn2 NeuronCores — full-device pipeline.

Sharding: (b, h) data/head parallel. Core c handles batch c//4, heads
{2*(c%4), 2*(c%4)+1}. Two Bass kernels per call:

  K1: qkv+rot projection (f32r), norms, f16 packing.  Host then does the
      tiny data-dependent part only: bucket argmax + stable argsort per
      (head, round) and assembles the packed gather payload + index
      tensors (all cheap numpy).
  K2: per (head, round): transposed dma_gather of the packed rows
      (sorted kn/v/OH/q), banded attention in a slot-mod-256 coordinate
      system (masks folded into PSUM via matmul accumulation with a big
      suppression constant B), exp (ACT) + divide (DVE), SBUF-source
      transpose-gather of P, PV with a ones-column producing Z for free,
      unsort via SBUF-source gather by rank, round-combine as
      sum(vo_unnorm)/sum(Z), and the output projection.

A pure-host fallback reproduces the same math if the device path fails.
"""
import os
import numpy as np

S, D, K, NB, CS, R, NH = 2048, 512, 64, 32, 64, 4, 8
N_CORES = 8
BSUP = float(2 ** 43)
SELF_VAL = -100000.0

_CACHE = {}


# ------------------------------------------------------------------ helpers
def _split_multi_waits(nc, max_waits=1):
    """Walrus in this env allows at most 1 sync wait per instruction.
    Move excess waits onto same-engine nops inserted just before."""
    import concourse.mybir as mybir
    uid = [0]
    for f in nc.m.functions:
        for bb in f.blocks:
            out = []
            for ins in bb.instructions:
                si = ins.sync_info
                waits = list(si.on_wait) if si and si.on_wait else []
                if len(waits) > max_waits:
                    extra, keep = waits[:-max_waits], waits[-max_waits:]
                    for k in range(0, len(extra), max_waits):
                        chunk = extra[k:k + max_waits]
                        uid[0] += 1
                        nop = mybir.InstNoOp(name=f"WS-{uid[0]}", ins=[], outs=[])
                        nop.engine = ins.engine
                        nop.sync_info = mybir.SyncInfo(on_wait=chunk, on_update=[])
                        out.append(nop)
                    si.on_wait = keep
                out.append(ins)
            bb.instructions = out


def _wrap_idx(v):
    """dma_gather idx layout: index j at [j%16, j//16], replicated to 128
    partitions (16-partition pattern x 8 gpsimd cores)."""
    w = np.ascontiguousarray(v.astype(np.int16).reshape(-1, 16).T)
    return np.tile(w, (8, 1))


# ------------------------------------------------------------------ K1
def _build_k1(split=True):
    import concourse.bass as bass
    import concourse.mybir as mybir
    from concourse.tile import TileContext

    dt = mybir.dt
    AF = mybir.ActivationFunctionType
    nc = bass.Bass(name="lsh_k1")
    xT = nc.dram_tensor("xT", [D, S], dt.float32r, kind="ExternalInput")
    wcat = nc.dram_tensor("wcat", [D, 384], dt.float32r, kind="ExternalInput")
    bcat = nc.dram_tensor("bcat", [1, 384], dt.float32r, kind="ExternalInput")
    knvq = nc.dram_tensor("knvq", [S, 384], dt.float16, kind="ExternalOutput")
    rot = nc.dram_tensor("rot", [S, 128], dt.float32, kind="ExternalOutput")

    with TileContext(nc) as tc:
        with (
            tc.tile_pool(name="wp", bufs=1) as wp,
            tc.tile_pool(name="xp", bufs=3) as xp,
            tc.tile_pool(name="sp", bufs=2) as sp,
            tc.tile_pool(name="pp", bufs=2, space="PSUM") as pp,
        ):
            w_sb = wp.tile([128, 4, 384], dt.float32r)
            nc.sync.dma_start(out=w_sb, in_=wcat[:, :].rearrange(
                "(kb p) n -> p kb n", p=128))
            b_sb = wp.tile([1, 384], dt.float32r)
            nc.sync.dma_start(out=b_sb, in_=bcat[:, :])
            ones1 = wp.tile([1, 128], dt.float32r)
            nc.vector.memset(ones1[:, :].bitcast(dt.float32), 1.0)

            for st in range(16):
                x_sb = xp.tile([128, 4, 128], dt.float32r, tag="x")
                for kb in range(4):
                    nc.sync.dma_start(
                        out=x_sb[:, kb, :],
                        in_=xT[kb * 128:(kb + 1) * 128, st * 128:(st + 1) * 128])
                qv = pp.tile([128, 512], dt.float32, tag="qv")
                nc.tensor.matmul(qv[:, 0:384], ones1, b_sb, start=True,
                                 stop=False)
                for kb in range(4):
                    nc.tensor.matmul(qv[:, 0:384], x_sb[:, kb, :],
                                     w_sb[:, kb, :],
                                     start=False, stop=(kb == 3))
                rt = sp.tile([128, 128], dt.float32, tag="rt")
                nc.scalar.activation(out=rt, in_=qv[:, 256:384], func=AF.Copy)
                nc.sync.dma_start(out=rot[st * 128:(st + 1) * 128, :], in_=rt)
                # norms
                sq = sp.tile([128, 128], dt.float32, tag="sq")
                nc.scalar.activation(out=sq, in_=qv[:, 0:128], func=AF.Square)
                nrm2 = sp.tile([128, 2], dt.float32, tag="n2")
                nc.vector.tensor_reduce(
                    out=nrm2, in_=sq[:, :].rearrange("p (h k) -> p h k", h=2),
                    axis=mybir.AxisListType.X, op=mybir.AluOpType.add)
                nrm = sp.tile([128, 2], dt.float32, tag="nr")
                nc.scalar.activation(out=nrm, in_=nrm2, func=AF.Sqrt)
                rn = sp.tile([128, 2], dt.float32, tag="rn")
                nc.vector.reciprocal(out=rn, in_=nrm)
                pk = sp.tile([128, 384], dt.float16, tag="pk")
                for h in range(2):
                    o = 192 * h
                    nc.vector.tensor_scalar(
                        out=pk[:, o:o + 64], in0=qv[:, 64 * h:64 * h + 64],
                        scalar1=rn[:, h:h + 1], scalar2=None,
                        op0=mybir.AluOpType.mult)
                    nc.scalar.activation(
                        out=pk[:, o + 64:o + 128],
                        in_=qv[:, 128 + 64 * h:192 + 64 * h], func=AF.Copy)
                    nc.vector.tensor_scalar(
                        out=pk[:, o + 128:o + 192],
                        in0=qv[:, 64 * h:64 * h + 64],
                        scalar1=0.125, scalar2=None, op0=mybir.AluOpType.mult)
                nc.sync.dma_start(out=knvq[st * 128:(st + 1) * 128, :], in_=pk)
    if split:
        _split_multi_waits(nc)
    return nc


# ------------------------------------------------------------------ K2
def _k2_pv_blocks(ch):
    """Valid (part0, part1, plane) PT blocks + VR slot base for chunk ch.
    o-coord: value of slot s sits at o = s mod 256; plane = o//128,
    partition = o%128. Returns list of (p0, p1, plane, slot_base)."""
    c = ch // 2
    m = ch % 4
    if m == 0:   # valid o: [192,256) + [0,128)
        return [(64, 128, 1, 128 * c - 64), (0, 128, 0, 128 * c)]
    if m == 1:   # [0,192)
        return [(0, 128, 0, 128 * c), (0, 64, 1, 128 * c + 128)]
    if m == 2:   # [64,256)
        return [(64, 128, 0, 128 * (c - 1) + 64), (0, 128, 1, 128 * c)]
    #  m == 3:   # [128,256) + [0,64)
    return [(0, 128, 1, 128 * c), (0, 64, 0, 128 * c + 128)]


def _k2_raw_pieces(c):
    """(o_start, length, slot_start) pieces covering psum cols for pair c."""
    if c % 2 == 0:
        return [(0, 192, (128 * c) % S), (192, 64, (128 * c - 64) % S)]
    return [(64, 192, (128 * c - 64) % S), (0, 64, (128 * c + 128) % S)]


def _build_k2(split=True, debug=False):
    import concourse.bass as bass
    import concourse.mybir as mybir
    from concourse.tile import TileContext

    dt = mybir.dt
    AF = mybir.ActivationFunctionType
    AL = mybir.AluOpType
    nc = bass.Bass(name="lsh_k2")
    pk = [nc.dram_tensor(f"pk{h}", [S, 384], dt.bfloat16, kind="ExternalInput")
          for h in range(2)]
    v1 = [nc.dram_tensor(f"v1{h}", [S, 128], dt.float16, kind="ExternalInput")
          for h in range(2)]
    idxs = nc.dram_tensor("idxs", [8, 128, 128], dt.int16, kind="ExternalInput")
    idxu = nc.dram_tensor("idxu", [8, 128, 128], dt.int16, kind="ExternalInput")
    idxi = nc.dram_tensor("idxi", [128, 128], dt.int16, kind="ExternalInput")
    wo2 = [nc.dram_tensor(f"wo2{h}", [64, 512], dt.float16, kind="ExternalInput")
           for h in range(2)]
    i21 = nc.dram_tensor("i21", [128, 128], dt.bfloat16, kind="ExternalInput")
    selfc = [nc.dram_tensor(f"self{p}", [128, 256], dt.bfloat16,
                            kind="ExternalInput") for p in range(2)]
    ot = nc.dram_tensor("ot", [512, S], dt.float16, kind="ExternalOutput")
    if debug:
        dbg_p = nc.dram_tensor("dbg_p", [128, 4096], dt.float16,
                               kind="ExternalOutput")
        dbg_vo = nc.dram_tensor("dbg_vo", [128, 2048], dt.float16,
                                kind="ExternalOutput")
        dbg_ut = nc.dram_tensor("dbg_ut", [128, 2048], dt.float16,
                                kind="ExternalOutput")
        dbg_raw = nc.dram_tensor("dbg_raw", [128, 1024], dt.float32,
                                 kind="ExternalOutput")
        dbg_dd = nc.dram_tensor("dbg_dd", [128, 1024], dt.float32,
                                kind="ExternalOutput")

    f16 = dt.float16

    with TileContext(nc) as tc:
        from concourse import library_config
        nc.gpsimd.load_library(library_config.mlp)
        with (
            tc.tile_pool(name="cp", bufs=1) as cp,
            tc.tile_pool(name="ip", bufs=2) as ip,
            tc.tile_pool(name="sg", bufs=2) as sgp,
            tc.tile_pool(name="vr", bufs=2) as vrp,
            tc.tile_pool(name="cb", bufs=2) as cbp,
            tc.tile_pool(name="ep", bufs=2) as epool,
            tc.tile_pool(name="pa", bufs=2) as pap,
            tc.tile_pool(name="pt", bufs=2) as ptp,
            tc.tile_pool(name="vo", bufs=2) as vop,
            tc.tile_pool(name="ut", bufs=2) as utp,
            tc.tile_pool(name="ac", bufs=1) as acp,
        ):
            i21_sb = cp.tile([128, 128], dt.bfloat16)
            nc.sync.dma_start(out=i21_sb, in_=i21[:, :])
            self_sb = []
            for p in range(2):
                t = cp.tile([128, 256], dt.bfloat16, name=f"selfsb{p}",
                            tag=f"selfsb{p}")
                nc.sync.dma_start(out=t, in_=selfc[p][:, :])
                self_sb.append(t)
            wo_sb = []
            for h in range(2):
                t = cp.tile([64, 512], f16, name=f"wosb{h}", tag=f"wosb{h}")
                nc.sync.dma_start(out=t, in_=wo2[h][:, :])
                wo_sb.append(t)
            ixi = cp.tile([128, 128], dt.int16)
            nc.sync.dma_start(out=ixi, in_=idxi[:, :])
            ones_t = cp.tile([128, 64], f16)
            nc.vector.memset(ones_t, 1.0)
            uts = [acp.tile([128, S], f16, tag=f"uts{h}", name=f"uts{h}")
                   for h in range(2)]

            jobs_psum = tc.tile_pool(name="ps", bufs=1, space="PSUM")
            psp = jobs_psum.__enter__()
            pv_psum = tc.tile_pool(name="pv", bufs=2, space="PSUM")
            pvp = pv_psum.__enter__()
            for jj in range(8):
                h, r = jj // 4, jj % 4
                ixs = ip.tile([128, 128], dt.int16, tag="ixs")
                nc.sync.dma_start(out=ixs, in_=idxs[jj, :, :])
                ixu = ip.tile([128, 128], dt.int16, tag="ixu")
                nc.sync.dma_start(out=ixu, in_=idxu[jj, :, :])

                sg = sgp.tile([128, 3, S], dt.bfloat16, tag="sg")
                nc.gpsimd.dma_gather(
                    out_ap=sg[:, :, :], in_ap=pk[h][:, :], idxs_ap=ixs[:, :],
                    num_idxs=S, num_idxs_reg=S, elem_size=384, transpose=True)
                vr = vrp.tile([128, 16, 128], f16, tag="vr")
                nc.gpsimd.dma_gather(
                    out_ap=vr[:, :, :], in_ap=v1[h][:, :], idxs_ap=ixs[:, :],
                    num_idxs=S, num_idxs_reg=S, elem_size=128, transpose=False)

                cbs = cbp.tile([128, S], dt.bfloat16, tag="cbs")
                rb = slice(32 * r, 32 * r + 32)
                sib = slice(32 * (r ^ 1), 32 * (r ^ 1) + 32)
                grp = slice(64 * (r // 2), 64 * (r // 2) + 64)
                nc.vector.memset(cbs[sib, :], 0.0)
                nc.vector.tensor_scalar(
                    out=cbs[rb, :], in0=sg[rb, 1, :], scalar1=-BSUP,
                    scalar2=BSUP, op0=AL.mult, op1=AL.add)

                p_all = pap.tile([128, 16, 256], f16, tag="pall")
                for g in range(4):
                    raw = psp.tile([128, 4, 256], dt.float32, tag="raw")
                    dd = psp.tile([128, 4, 256], dt.float32, tag="dd")
                    for c4 in range(4):
                        c = 4 * g + c4
                        qs = slice(128 * c, 128 * c + 128)
                        qT = sg[0:64, 2, qs].bitcast(f16)
                        knT = sg[0:64, 0, :].bitcast(f16)
                        first_bank = (c4 % 2 == 0)
                        last_bank = (c4 % 2 == 1)
                        for pi, (o0, ln, s0) in enumerate(_k2_raw_pieces(c)):
                            ws = slice(s0, s0 + ln)
                            nc.tensor.matmul(
                                raw[:, c4, o0:o0 + ln], qT, knT[:, ws],
                                start=(first_bank and pi == 0),
                                stop=(last_bank and pi == 1),
                                skip_group_check=True)
                            nc.tensor.matmul(
                                dd[:, c4, o0:o0 + ln], sg[:, 1, qs],
                                sg[:, 1, ws],
                                start=(first_bank and pi == 0), stop=False,
                                skip_group_check=True)
                            nc.tensor.matmul(
                                dd[:, c4, o0:o0 + ln], sg[grp, 1, qs],
                                cbs[grp, ws], start=False, stop=False,
                                skip_group_check=True)
                        nc.tensor.matmul(
                            dd[:, c4, :], i21_sb, self_sb[c % 2],
                            start=False, stop=last_bank,
                            skip_group_check=True)
                    et = epool.tile([128, 4, 256], f16, tag="et")
                    nc.scalar.activation(
                        out=et[:, :, :].rearrange("p a b -> p (a b)"),
                        in_=raw[:, :, :].rearrange("p a b -> p (a b)"),
                        func=AF.Exp)
                    nc.vector.tensor_tensor(
                        out=p_all[:, 4 * g:4 * g + 4, :].rearrange(
                            "p a b -> p (a b)"),
                        in0=et[:, :, :].rearrange("p a b -> p (a b)"),
                        in1=dd[:, :, :].rearrange("p a b -> p (a b)"),
                        op=AL.divide)
                    if debug and jj == 0 and g == 0:
                        rawc = epool.tile([128, 1024], dt.float32, tag="rawc")
                        nc.scalar.activation(
                            out=rawc,
                            in_=raw[:, :, :].rearrange("p a b -> p (a b)"),
                            func=AF.Copy)
                        nc.sync.dma_start(out=dbg_raw[:, :], in_=rawc)
                        ddc = epool.tile([128, 1024], dt.float32, tag="ddc")
                        nc.scalar.activation(
                            out=ddc,
                            in_=dd[:, :, :].rearrange("p a b -> p (a b)"),
                            func=AF.Copy)
                        nc.sync.dma_start(out=dbg_dd[:, :], in_=ddc)

                pt = ptp.tile([128, 2, S], f16, tag="pt")
                nc.gpsimd.dma_gather(
                    out_ap=pt[:, :, :], in_ap=p_all[:, :, :], idxs_ap=ixi[:, :],
                    num_idxs=S, num_idxs_reg=S, elem_size=256, transpose=True,
                    sbuf_tokens_per_rank=128, sbuf_free_dim_per_rank=512)

                vo = vop.tile([128, 16, 128], f16, tag="vo")
                nc.vector.memset(vo[:, :, 65:128], 0.0)
                for g in range(4):
                    pv = pvp.tile([128, 4, 128], dt.float32, tag="pv")
                    for c4 in range(4):
                        c = 4 * g + c4
                        for half in range(2):
                            ch = 2 * c + half
                            qs2 = slice(64 * ch, 64 * ch + 64)
                            rows = slice(64 * half, 64 * half + 64)
                            blocks = _k2_pv_blocks(ch)
                            for bi, (p0, p1, pl, sb0) in enumerate(blocks):
                                gslot = ((sb0 % S) // 128)
                                nc.tensor.matmul(
                                    pv[rows, c4, 0:65], pt[p0:p1, pl, qs2],
                                    vr[p0:p1, gslot, 0:65],
                                    start=(c4 == 0 and bi == 0),
                                    stop=(c4 == 3 and half == 1
                                          and bi == len(blocks) - 1),
                                    skip_group_check=True)
                    nc.vector.tensor_copy(
                        out=vo[:, 4 * g:4 * g + 4, 0:65], in_=pv[:, :, 0:65])

                if debug and jj == 0:
                    nc.sync.dma_start(
                        out=dbg_p[:, :],
                        in_=p_all[:, 0:16, :].rearrange("p a b -> p (a b)"))
                    nc.sync.dma_start(
                        out=dbg_vo[:, :],
                        in_=vo[:, :, :].rearrange("p a b -> p (a b)"))
                ut = utp.tile([128, 1, S], f16, tag="ut")
                nc.gpsimd.dma_gather(
                    out_ap=ut[:, :, :], in_ap=vo[:, :, :], idxs_ap=ixu[:, :],
                    num_idxs=S, num_idxs_reg=S, elem_size=128, transpose=True,
                    sbuf_tokens_per_rank=128, sbuf_free_dim_per_rank=256)
                if debug and jj == 0:
                    nc.sync.dma_start(out=dbg_ut[:, :], in_=ut[:, 0, :])
                if r == 0:
                    nc.vector.tensor_copy(out=uts[h], in_=ut[:, 0, :])
                else:
                    nc.vector.tensor_tensor(
                        out=uts[h], in0=uts[h], in1=ut[:, 0, :], op=AL.add)

            pv_psum.__exit__(None, None, None)
            jobs_psum.__exit__(None, None, None)

            zb_psum = tc.tile_pool(name="zb", bufs=2, space="PSUM")
            zbp = zb_psum.__enter__()
            mids = []
            for h in range(2):
                mid = acp.tile([64, S], f16, tag=f"mid{h}", name=f"mid{h}")
                for half in range(2):
                    zb = zbp.tile([64, 2, 512], dt.float32, tag="zb")
                    for i in range(2):
                        sc0 = 1024 * half + 512 * i
                        nc.tensor.matmul(
                            zb[:, i, :], ones_t[64:65, :],
                            uts[h][64:65, sc0:sc0 + 512],
                            start=True, stop=True)
                    nc.vector.tensor_tensor(
                        out=mid[:, 1024 * half:1024 * half + 1024],
                        in0=uts[h][0:64, 1024 * half:1024 * half + 1024],
                        in1=zb[:, :, :].rearrange("p a b -> p (a b)"),
                        op=AL.divide)
                mids.append(mid)
            zb_psum.__exit__(None, None, None)
            out_psum = tc.tile_pool(name="po", bufs=2, space="PSUM")
            pop = out_psum.__enter__()

            for ob in range(4):
                for sc in range(4):
                    otp = pop.tile([128, 512], dt.float32, tag="otp")
                    for h in range(2):
                        nc.tensor.matmul(
                            otp, wo_sb[h][:, 128 * ob:128 * ob + 128],
                            mids[h][:, 512 * sc:512 * sc + 512],
                            start=(h == 0), stop=(h == 1))
                    ots = utp.tile([128, 512], f16, tag="ots")
                    nc.vector.tensor_copy(out=ots, in_=otp)
                    nc.sync.dma_start(
                        out=ot[128 * ob:128 * ob + 128,
                               512 * sc:512 * sc + 512], in_=ots)
            out_psum.__exit__(None, None, None)
    if split:
        _split_multi_waits(nc)
    return nc


# ------------------------------------------------------------------ host prep
def _host_prep_core(core, x, Wq, bq, Wv, bv, Wo, bo, hash_vec):
    cb, h0 = core // 4, 2 * (core % 4)
    wcols, bcols = [], []
    for fam, Wf, bf in (("q", Wq, bq), ("v", Wv, bv)):
        for h in (h0, h0 + 1):
            wcols.append(Wf[:, 64 * h:64 * h + 64])
            bcols.append(bf[64 * h:64 * h + 64])
    for h in (h0, h0 + 1):
        H = hash_vec[h].reshape(64, 64)
        wcols.append(Wq[:, 64 * h:64 * h + 64] @ H)
        bcols.append(bq[64 * h:64 * h + 64] @ H)
    wcat = np.ascontiguousarray(np.concatenate(wcols, axis=1), np.float32)
    bcat = np.ascontiguousarray(
        np.concatenate(bcols).reshape(1, 384), np.float32)
    xT = np.ascontiguousarray(x[cb].T, np.float32)
    return {"xT": xT, "wcat": wcat, "bcat": bcat}


def _host_middle_core(core, knvq, rot, Wo):
    """Build K2 inputs from K1 outputs (numpy)."""
    import ml_dtypes
    h0 = 2 * (core % 4)
    ar = np.arange(S)
    out = {}
    for hh in range(2):
        rotm = rot[:, 64 * hh:64 * hh + 64].reshape(S, 16, 4)
        cat = np.concatenate([-rotm, rotm], axis=1)       # (S, 32, 4)
        bk = np.argmax(cat, axis=1)                       # (S, 4)
        oh = (bk[:, None, :] == np.arange(32)[None, :, None])  # (S, 32, 4)
        oh128 = np.ascontiguousarray(
            oh.transpose(0, 2, 1).reshape(S, 128))        # r-major blocks
        base = 192 * hh
        kn = knvq[:, base:base + 64]
        v = knvq[:, base + 64:base + 128]
        q = knvq[:, base + 128:base + 192]
        pkbits = np.zeros((S, 384), np.uint16)
        pkbits[:, 0:64] = np.ascontiguousarray(kn).view(np.uint16)
        pkbits[:, 64:128] = np.ascontiguousarray(v).view(np.uint16)
        pkbits[:, 128:256] = np.where(oh128, np.uint16(0x3F80), np.uint16(0))
        pkbits[:, 256:320] = np.ascontiguousarray(q).view(np.uint16)
        out[f"pk{hh}"] = pkbits.view(ml_dtypes.bfloat16)
        v1b = np.zeros((S, 128), np.float16)
        v1b[:, 0:64] = v
        v1b[:, 64] = 1.0
        out[f"v1{hh}"] = v1b
        for r in range(4):
            key = bk[:, r].astype(np.int64) * S + ar
            st = np.argsort(key, kind="stable")
            rank = np.argsort(st, kind="stable")
            out.setdefault("idxs", np.zeros((8, 128, 128), np.int16))[
                4 * hh + r] = _wrap_idx(st)
            out.setdefault("idxu", np.zeros((8, 128, 128), np.int16))[
                4 * hh + r] = _wrap_idx(rank)
        out[f"wo2{hh}"] = np.ascontiguousarray(
            Wo[64 * (h0 + hh):64 * (h0 + hh) + 64, :]).astype(np.float16)
    out["idxi"] = _wrap_idx(ar)
    i21 = (np.eye(128, dtype=np.float32) * float(2 ** 21))
    out["i21"] = i21.astype(ml_dtypes.bfloat16)
    se = np.zeros((128, 256), np.float32)
    se[np.arange(128), np.arange(128)] = float(2 ** 22)
    so = np.zeros((128, 256), np.float32)
    so[np.arange(128), 128 + np.arange(128)] = float(2 ** 22)
    out["self0"] = se.astype(ml_dtypes.bfloat16)
    out["self1"] = so.astype(ml_dtypes.bfloat16)
    return out


def _get_nc(key, builder):
    if key not in _CACHE:
        _CACHE[key] = builder()
    return _CACHE[key]


def _run_spmd(nc, in_maps):
    from concourse.bass_utils import run_bass_kernel_spmd
    res = run_bass_kernel_spmd(nc, in_maps, core_ids=list(range(N_CORES)))
    return res.results


# ------------------------------------------------------------------ fallback
def _host_fallback(x, Wq, bq, Wv, bv, Wo, bo, hash_vec):
    b = x.shape[0]
    out = np.zeros((b, S, D), np.float32)
    for core in range(N_CORES):
        cb, h0 = core // 4, 2 * (core % 4)
        acc = np.zeros((S, D), np.float32)
        for hh in range(2):
            h = h0 + hh
            qk = x[cb] @ Wq[:, 64 * h:64 * h + 64] + bq[64 * h:64 * h + 64]
            v = x[cb] @ Wv[:, 64 * h:64 * h + 64] + bv[64 * h:64 * h + 64]
            H = hash_vec[h].reshape(64, 64)
            rot = qk @ H
            rotm = rot.reshape(S, 16, 4)
            cat = np.concatenate([-rotm, rotm], axis=1)
            bk = np.argmax(cat, axis=1)
            nrm = np.maximum(np.sqrt((qk * qk).sum(1, keepdims=True)), 1e-12)
            kn = (qk / nrm).astype(np.float32)
            q = (qk / 8.0).astype(np.float32)
            oh = (bk[:, None, :] == np.arange(32)[None, :, None]).astype(
                np.float32).transpose(0, 2, 1).reshape(S, 128)
            svo = np.zeros((S, 64), np.float32)
            sz = np.zeros((S,), np.float32)
            ar = np.arange(S)
            for r in range(4):
                key = bk[:, r].astype(np.int64) * S + ar
                st = np.argsort(key, kind="stable")
                sq_, sk, sv_, soh = q[st], kn[st], v[st], oh[st]
                vo_u = np.zeros((S, 64), np.float32)
                zz = np.zeros((S,), np.float32)
                for c in range(NB):
                    qs = slice(64 * c, 64 * c + 64)
                    kidx = np.arange(64 * (c - 1), 64 * (c + 2)) % S
                    dots = sq_[qs] @ sk[kidx].T
                    dup = soh[qs] @ soh[kidx].T
                    sb = (soh[qs, 32 * r:32 * r + 32]
                          @ soh[kidx, 32 * r:32 * r + 32].T)
                    dd = dup + BSUP * (1.0 - sb)
                    dd[np.arange(64), 64 + np.arange(64)] += BSUP
                    p = np.exp(dots) / dd
                    vo_u[qs] = p @ sv_[kidx]
                    zz[qs] = p.sum(1)
                dest = np.argsort(st, kind="stable")
                svo += vo_u[dest]
                sz += zz[dest]
            mid = svo / np.maximum(sz, 1e-30)[:, None]
            acc += mid @ Wo[64 * h:64 * h + 64, :]
        out[cb] += acc
    out += bo[None, None, :]
    return out


# ------------------------------------------------------------------ entry
def kernel(x, Wq, bq, Wv, bv, Wo, bo, hash_vec):
    x = np.asarray(x, np.float32)
    Wq, bq = np.asarray(Wq, np.float32), np.asarray(bq, np.float32)
    Wv, bv = np.asarray(Wv, np.float32), np.asarray(bv, np.float32)
    Wo, bo = np.asarray(Wo, np.float32), np.asarray(bo, np.float32)
    hash_vec = np.asarray(hash_vec, np.float32)
    if os.environ.get("KERNEL_NO_DEVICE"):
        return _host_fallback(x, Wq, bq, Wv, bv, Wo, bo, hash_vec)
    try:
        in1 = [_host_prep_core(c, x, Wq, bq, Wv, bv, Wo, bo, hash_vec)
               for c in range(N_CORES)]
        r1 = _run_spmd(_get_nc("k1", _build_k1), in1)
        in2 = [_host_middle_core(c, np.asarray(r1[c]["knvq"]),
                                 np.asarray(r1[c]["rot"]), Wo)
               for c in range(N_CORES)]
        r2 = _run_spmd(_get_nc("k2", _build_k2), in2)
        out = np.zeros((x.shape[0], S, D), np.float32)
        for c in range(N_CORES):
            out[c // 4] += np.asarray(r2[c]["ot"], np.float32).T
        out += bo[None, None, :]
        return out
    except Exception:
        import traceback
        traceback.print_exc()
        return _host_fallback(x, Wq, bq, Wv, bv, Wo, bo, hash_vec)
